# revision 34
# baseline (speedup 1.0000x reference)
"""ClothLinearFusion Trainium2 kernel.

Computes out[b, i] = (sum_k cloth[b, k, i]) * (sum_j f[i, j] * body[b, j])
for cloth (128, 64, 1024), body (128, 1024), f (1024, 1024), all fp32.

Sharding: split the cloth-channel dim C=1024 into 8 slices of 128, one per
NeuronCore. Each core reads its cloth slice (4 MB), its slice of f.T
(0.5 MB) and the full body.T (0.5 MB) — 5 MB/core, vs 8 MB/core for
batch-parallel sharding (which would replicate all of f). Outputs
(128, 128) per core are concatenated on the channel axis.

Host-side prep (numpy, layout only): per core, cloth is cut into k-chunks;
bf = concat([body.T, fT_slice], axis=1) is swizzled to [p, jchunk, 256]
(contraction dim j on SBUF partitions — PE contracts over partitions and
fp32 has no on-chip DMA-transpose) and folded INTO the cloth chunk arrays:
chunks 1..4 each carry 2 j-chunks (512 floats) appended per partition, so
bf needs no separate transfer and never bubbles the cloth stream.

Schedule: all bulk DMAs ride the single qSPDynamicHW ring in strict FIFO
(chunk 0 rides the otherwise-idle ACT ring to start ~1.5 us earlier);
each arriving chunk is tree-reduced over k on DVE (fp32 tensor_tensor is
1 elem/cycle/lane — the binary tree is the cheapest form) into a running
accumulator; the 8 fp32 matmuls accumulate fv in PSUM as their operand
chunks land; one PSUM->SBUF copy + elementwise mul + store finish.

Measured: ~24-26 us HW exec at the chip's fast clock state (best 23.9);
runs land at 28-32 us when the chip sits in a ~1.2x slower clock state or
when SDMA engine 15 sporadically lags ~25% (it gates every transfer's
completion semaphore) — both environmental. DMA stream floor is ~12.5 us
at ~430 GB/s for the 5.0 MB/core; the ~12.5 us DVE reduction (fp32 TT is
hard-capped at 1 elem/cycle/lane) overlaps it almost fully; ~4.3 us of
NRT per-engine preamble inside the exec window is fixed cost.
"""

import sys

sys.path.insert(0, "/opt/trn_rl_repo")

import numpy as np

import bass_rust
import concourse.bass as bass
import concourse.mybir as mybir
import concourse.tile as tile
from concourse.bass_utils import run_bass_kernel_spmd
from concourse.vector_clock import ScopedClock

B = 128          # batch
K = 64           # cloth latent count (summed away)
C = 1024         # cloth channels
J = 1024         # body channels
NCORES = 8
CI = C // NCORES  # cloth channels per core = 128
# k-chunk sizes: small first (starts the DVE pipeline early), big while
# streaming, small tail. Total DMA count (chunks + out) must stay <= 8
# so no DMAHW sem lane is reused (lane reuse adds a second sync wait, which
# this walrus rejects). The first NBF chunks each carry 2 j-chunks of the
# bf matmul operands appended per partition (512 floats), so bf needs no
# separate transfer and never bubbles the cloth stream.
KCHUNKS = [2, 16, 16, 16, 8, 4, 2]
NBF = 4          # chunks that carry bf pieces
BFW = 512        # floats of bf payload per partition per carrying chunk

F32 = mybir.dt.float32

_CACHE = {}


# ---------------------------------------------------------------------------
# Framework patches for this container's walrus (ONE sync wait per
# instruction) and slow GpSimd teardown.
# ---------------------------------------------------------------------------

def _split_drain_and_barrier(self, tick_clock, wait_clock):
    """TileContext._drain_and_barrier with the multi-sem wait split into one
    drain per semaphore (walrus here rejects >1 sync wait per instruction)."""
    nc = self.nc
    drain_inst = nc.sync.drain()
    wait_clock.add_sem_waits(
        drain_inst.ins, ScopedClock({None: tick_clock.global_clock})
    )
    si = drain_inst.ins.sync_info
    if si is not None and len(si.on_wait) > 1:
        waits = list(si.on_wait)
        drain_inst.ins.sync_info = bass_rust.SyncInfo(
            on_wait=waits[:1], on_update=list(si.on_update)
        )
        for w in waits[1:]:
            extra = nc.sync.drain()
            extra.ins.sync_info = bass_rust.SyncInfo(on_wait=[w], on_update=[])

    # sem_only: the stock barrier drains every engine, and a Pool (Q7) drain
    # costs ~3.4 us; the split drains above already wait for all work.
    nc.all_engine_barrier(sem_only=True)
    assert self.sems is not None
    popped = nc._tile_sem_poison_stack.pop()
    assert popped is self._sem_poison
    nc.clear_and_free_semaphores(list(self.sems.allocated().values()))
    nc.all_engine_barrier(sem_only=True)


tile.TileContext._drain_and_barrier = _split_drain_and_barrier


def _compact_to_ranges(nums):
    nums = sorted(set(nums))
    ranges = []
    start = prev = nums[0]
    for n in nums[1:]:
        if n == prev + 1:
            prev = n
            continue
        ranges.append(range(start, prev + 1))
        start = prev = n
    ranges.append(range(start, prev + 1))
    return ranges


def _fast_clear_and_free_semaphores(self, sems):
    """Bass.clear_and_free_semaphores via SP instead of GpSimd — the Q7
    dma_reset + sem_clear pair costs ~3.5 us each on Pool."""
    if not sems:
        return
    sem_nums = [s.num if hasattr(s, "num") else s for s in sems]
    for sem_range in _compact_to_ranges(sem_nums):
        assert self._state.free_isdisjoint(sem_range)
        self.sync.drain(semaphore_range=sem_range)
        self.sync.sem_clear(sem_range)
    self._state.prepend_free_semaphores(sem_nums)
    for poison_set in self._tile_sem_poison_stack:
        poison_set.update(sem_nums)


def _strip_preamble(nc):
    """Remove the const-AP memsets (unused here; ~3.5 us of GpSimd time) and
    the initial all-engine barrier from the Bass preamble. Cross-engine
    ordering inside the kernel body is fully sem-managed by Tile."""
    main_blk = None
    for fn in nc.m.functions:
        for blk in fn.blocks:
            if blk.name == "main":
                main_blk = blk
    assert main_blk is not None
    to_drop = []
    for inst in main_blk.instructions:
        t = type(inst).__name__
        if t == "InstMemset":
            to_drop.append(inst)
        elif t in ("InstDrain", "InstEventSemaphore"):
            to_drop.append(inst)
    for inst in to_drop:
        main_blk.instructions.remove(inst)


def _assert_single_waits(nc):
    for fn in nc.m.functions:
        for blk in fn.blocks:
            for inst in blk.instructions:
                si = inst.sync_info
                if si is not None and len(si.on_wait) > 1:
                    raise AssertionError(
                        f"{type(inst).__name__} {inst.name} has "
                        f"{len(si.on_wait)} waits: "
                        f"{[(w.ant_name, w.wait_value) for w in si.on_wait]}"
                    )


# ---------------------------------------------------------------------------
# Kernel program (SPMD, identical on all 8 cores)
# ---------------------------------------------------------------------------

def _build_program():
    nc = bass.Bass(target_bir_lowering=False, debug=False)
    nc.clear_and_free_semaphores = _fast_clear_and_free_semaphores.__get__(nc)

    # chunk q (q < NBF): per partition [ks*CI floats cloth | 512 floats bf]
    # where the bf payload is j-chunks 2q, 2q+1 of [bodyT | fT_slice].
    ins = []
    for q, ks in enumerate(KCHUNKS):
        w = ks * CI + (BFW if 1 <= q <= NBF else 0)
        ins.append(nc.dram_tensor(f"in{q}", [B, w], F32, kind="ExternalInput"))
    out = nc.dram_tensor("out_s", [B, CI], F32, kind="ExternalOutput")

    JCH = J // 128

    with tile.TileContext(nc) as tc:
        with (
            tc.tile_pool(name="pool", bufs=1) as pool,
            tc.tile_pool(name="tree", bufs=2) as tree_pool,
            tc.tile_pool(name="psum", bufs=1, space=bass.MemorySpace.PSUM) as psum_pool,
        ):
            # --- DMA issue order == qSPDynamicHW FIFO order ---
            # chunk 0 rides the otherwise-empty ACT HWDGE ring: its packets
            # flow while SP is still in its runtime preamble / first issue
            # (the ACT ring is only starved while the SP ring has work).
            chunks = []
            for q, ks in enumerate(KCHUNKS):
                w = ks * CI + (BFW if 1 <= q <= NBF else 0)
                ch = pool.tile([B, w], F32, tag=f"ch{q}")
                eng = nc.scalar if q == 0 else nc.sync
                eng.dma_start(out=ch[:], in_=ins[q][:])
                chunks.append((ch, ks))

            # --- fv[b, ci] = sum_j body[b, j] * f[ci, j] on PE ---
            # j-chunk c rides cloth chunk c//2; matmuls pipeline with arrivals
            fv_psum = psum_pool.tile([B, CI], F32)
            for c in range(JCH):
                ch, ks = chunks[c // 2 + 1]
                base = ks * CI + (c % 2) * 256
                nc.tensor.matmul(
                    fv_psum[:],
                    ch[:, base:base + B],
                    ch[:, base + B:base + B + CI],
                    start=(c == 0),
                    stop=(c == JCH - 1),
                )

            # --- c_sum via DVE binary-tree adds, chunk-pipelined ---
            acc = pool.tile([B, CI], F32)
            fv_sb = pool.tile([B, CI], F32)
            for q, (ch, ks) in enumerate(chunks):
                cur = ch[:, 0:ks * CI].rearrange("p (k n) -> p k n", n=CI)
                n = ks
                leftovers = []  # odd-level remainders, folded into acc below
                while n > 2:
                    half = n // 2
                    if n % 2:
                        leftovers.append(cur[:, n - 1, :])
                    t = tree_pool.tile([B, half, CI], F32, tag=f"t{q}_{half}")
                    nc.vector.tensor_add(
                        out=t[:], in0=cur[:, 0:half, :], in1=cur[:, half:2 * half, :]
                    )
                    cur, n = t[:], half
                # last level writes the chunk partial (unique tag: a shared
                # slot would add a WAR wait on top of the DMA wait)
                partial = tree_pool.tile([B, CI], F32, tag=f"partial{q}")
                if n == 2:
                    nc.vector.tensor_add(
                        out=partial[:], in0=cur[:, 0, :], in1=cur[:, 1, :]
                    )
                else:
                    nc.vector.tensor_copy(out=partial[:], in_=cur[:, 0, :])
                if q == 0:
                    first_partial = partial
                elif q == 1:
                    nc.vector.tensor_add(
                        out=acc[:], in0=first_partial[:], in1=partial[:]
                    )
                else:
                    nc.vector.tensor_add(out=acc[:], in0=acc[:], in1=partial[:])
                for lo in leftovers:
                    nc.vector.tensor_add(out=acc[:], in0=acc[:], in1=lo)
                if q == 4:
                    # DVE slack while the tail chunks stream: PSUM -> SBUF
                    # copy of fv (single PE wait; keeps the mul single-wait)
                    nc.vector.tensor_copy(out=fv_sb[:], in_=fv_psum[:])

            # --- out = c_sum * fv ---
            res = pool.tile([B, CI], F32)
            nc.vector.tensor_mul(out=res[:], in0=acc[:], in1=fv_sb[:])
            nc.sync.dma_start(out=out[:], in_=res[:])

    _strip_preamble(nc)
    _assert_single_waits(nc)
    return nc


def _get_program():
    if "nc" not in _CACHE:
        _CACHE["nc"] = _build_program()
    return _CACHE["nc"]


def _make_in_maps(cloth_latent, body_latent, f):
    cloth_latent = np.asarray(cloth_latent, dtype=np.float32)
    body_latent = np.asarray(body_latent, dtype=np.float32)
    f = np.asarray(f, dtype=np.float32)

    bodyT = body_latent.T                                # (J, B) view
    fT = f.T                                             # (J, C) view

    in_maps = []
    for i in range(NCORES):
        sl = slice(i * CI, (i + 1) * CI)
        bf = np.concatenate([bodyT, fT[:, sl]], axis=1)  # (J, B + CI)
        # swizzle to [p, jchunk, B+CI]: row j = c*128 + p
        bf_r = bf.reshape(J // 128, 128, B + CI).transpose(1, 0, 2)  # (128, 8, 256)
        cl = cloth_latent[:, :, sl]                      # (B, K, CI) view

        m = {}
        k0 = 0
        for q, ks in enumerate(KCHUNKS):
            cpart = cl[:, k0:k0 + ks, :].reshape(B, ks * CI)
            if 1 <= q <= NBF:
                bpart = bf_r[:, 2 * (q - 1):2 * q, :].reshape(B, BFW)
                m[f"in{q}"] = np.ascontiguousarray(
                    np.concatenate([cpart, bpart], axis=1)
                )
            else:
                m[f"in{q}"] = np.ascontiguousarray(cpart)
            k0 += ks
        in_maps.append(m)
    return in_maps


def _run(cloth_latent, body_latent, f, trace=False):
    nc = _get_program()
    in_maps = _make_in_maps(cloth_latent, body_latent, f)
    r = run_bass_kernel_spmd(nc, in_maps, list(range(NCORES)), trace=trace)
    out = np.concatenate([r.results[i]["out_s"] for i in range(NCORES)], axis=1)
    return out, r


def kernel(cloth_latent, body_latent, f):
    out, _ = _run(cloth_latent, body_latent, f, trace=False)
    return out


def kernel_traced(cloth_latent, body_latent, f):
    """Returns (output, BassKernelResults) with NTFF profiling enabled."""
    return _run(cloth_latent, body_latent, f, trace=True)
